# revision 28
# baseline (speedup 1.0000x reference)
"""Causal multi-head attention block (qkv proj + attention + out proj) on 8
Trainium2 NeuronCores.

Sharding: Megatron-style tensor parallel over heads — 2 heads per core.
Each core computes its heads' Q/K/V projections (column-sharded w_qkv),
causal attention for those heads, and a row-sharded partial of the output
projection.  The host sums the 8 partial outputs and adds b_o.

Device-side layout notes:
 - The host feeds X^T [C, B*T] (bf16) so every matmul contraction dim (C,
   head dim, or key position) lands on SBUF partitions with no on-device
   transposes of activations.  Scores are computed transposed
   (S^T[k, q] = K^T.T @ Q^T per 128-wide k block), softmax-exp runs on the
   scalar engine, and the denominator is produced by an extra all-ones
   column appended to V (row 64 of the attn@V accumulator).
 - V^T -> V goes through the XBAR dma transpose path (16x128 tiles) into a
   contiguous scratch, then one DVE copy interleaves each head into the
   [V_h0 | 1 | V_h1 | 1] augmented layout.  The ones columns are memset
   once into two persistent v buffers (manual double buffer across batches).
 - The whole kernel is software-pipelined at emission level: the attention
   phase of batch b is exp(ACT)-paced, so stage A (qkv projection) of batch
   b+1, the output projection, and the softmax-divide muls are chopped into
   small units and dealt one per kb-block into the attention instruction
   stream.  This keeps the in-order PE queue fed (no engine ever waits
   long on a cross-engine dependency) and the PE clock un-throttled.
"""

import numpy as np
import ml_dtypes
from collections import deque

import concourse.bass as bass
import concourse.tile as tile
import concourse.mybir as mybir
from concourse import bacc
from concourse.bass_utils import run_bass_kernel_spmd

B, T, C, H, DH = 4, 2048, 1024, 16, 64
NCORES = 8
HPC = H // NCORES            # heads per core = 2
R = B * T                    # 8192 rows
HD = HPC * DH                # 128 local head dims
KT = C // 128                # 8 contraction tiles over C
RC = 512                     # row chunk in qkv stage
QC = 512                     # query chunk in attention
NQC = T // QC                # 4
NKB = T // 128               # 16 key blocks

F32 = mybir.dt.float32
BF16 = mybir.dt.bfloat16

LAST_RESULT = None           # BassKernelResults of the most recent run
_CACHED_NC = None


def _emit(nc, tc, xt, wqkv, bqkv, wo, tri, y, use_bias=True):
    from contextlib import ExitStack

    Exp = mybir.ActivationFunctionType.Exp
    with ExitStack() as ctx:
        const = ctx.enter_context(tc.tile_pool(name="const", bufs=1))
        bigp = ctx.enter_context(tc.tile_pool(name="bigp", bufs=2))
        xtp = ctx.enter_context(tc.tile_pool(name="xtp", bufs=4))
        vtp = ctx.enter_context(tc.tile_pool(name="vtp", bufs=2))
        vscrp = ctx.enter_context(tc.tile_pool(name="vscrp", bufs=2))
        ptp = ctx.enter_context(tc.tile_pool(name="ptp", bufs=7))
        osbp = ctx.enter_context(tc.tile_pool(name="osbp", bufs=2))
        ystp = ctx.enter_context(tc.tile_pool(name="ystp", bufs=4))
        smallp = ctx.enter_context(tc.tile_pool(name="smallp", bufs=3))
        onump = ctx.enter_context(tc.tile_pool(name="onump", bufs=3))
        psA = ctx.enter_context(tc.tile_pool(name="psA", bufs=2, space="PSUM"))
        psS = ctx.enter_context(tc.tile_pool(name="psS", bufs=2, space="PSUM"))
        psO = ctx.enter_context(tc.tile_pool(name="psO", bufs=1, space="PSUM"))

        # ---- constants ----
        w_sb = const.tile([128, KT, 3 * HD], BF16, name="w_sb")
        wqkv_r = wqkv[:, :].rearrange("(ko ki) m -> ki ko m", ki=128)
        for m in range(3):
            # split per q/k/v section so the first qkv matmul group only
            # waits on a third of the weight bytes
            nc.sync.dma_start(
                out=w_sb[:, :, m * HD : (m + 1) * HD],
                in_=wqkv_r[:, :, m * HD : (m + 1) * HD],
            )
        wo_sb = const.tile([128, C], BF16, name="wo_sb")
        nc.sync.dma_start(out=wo_sb[:, :], in_=wo[:, :])
        b_sb = const.tile([128, 3], F32, name="b_sb")
        for m in range(3):
            nc.sync.dma_start(
                out=b_sb[:, m : m + 1],
                in_=bqkv[m : m + 1, :].rearrange("a n -> n a"),
            )
        tri_sb = const.tile([128, 128], BF16, name="tri_sb")
        nc.sync.dma_start(out=tri_sb[:, :], in_=tri[:, :])

        # persistent double-buffered V_aug tiles; ones columns written once
        v_bufs = [
            const.tile([128, NKB, 130], BF16, name=f"v_sb{i}") for i in range(2)
        ]
        for vb in v_bufs:
            nc.vector.memset(vb[:, :, 64:65], 1.0)
            nc.vector.memset(vb[:, :, 129:130], 1.0)

        # warm up the ACT exp table while stage A of batch 0 runs
        warm = const.tile([1, 1], F32, name="warm")
        nc.vector.memset(warm[:, :], 0.0)
        nc.scalar.activation(out=warm[:, :], in_=warm[:, :], func=Exp)

        xt_r = xt[:, :].rearrange("(ko ki) n -> ki ko n", ki=128)

        # ---- stage A of one batch, chopped into pump-able steps ----
        def make_stage_a(bb):
            qt = bigp.tile([128, T], BF16, name="qt", tag="qt")
            ktt = bigp.tile([128, T], BF16, name="ktt", tag="ktt")
            vt_b = vtp.tile([128, T], BF16, name="vt_b", tag="vt")
            xts = {}

            def dma_step(rcl):
                def f():
                    x_t = xtp.tile([128, KT, RC], BF16, name="x_t", tag="xt")
                    rc = bb * (T // RC) + rcl
                    nc.sync.dma_start(
                        out=x_t[:, :, :],
                        in_=xt_r[:, :, rc * RC : (rc + 1) * RC],
                    )
                    xts[rcl] = x_t
                return f

            def mm_step(rcl, m):
                def f():
                    ps = psA.tile([128, RC], F32, name="ps_qkv", tag="qkv")
                    for k in range(KT):
                        nc.tensor.matmul(
                            ps[:, :],
                            lhsT=w_sb[:, k, m * HD : (m + 1) * HD],
                            rhs=xts[rcl][:, k, :],
                            start=(k == 0),
                            stop=(k == KT - 1),
                        )
                    if m == 0:
                        dst = qt[:, rcl * RC : (rcl + 1) * RC]
                    elif m == 1:
                        dst = ktt[:, rcl * RC : (rcl + 1) * RC]
                    else:
                        dst = vt_b[:, rcl * RC : (rcl + 1) * RC]
                    if use_bias:
                        nc.vector.tensor_scalar_add(
                            out=dst, in0=ps[:, :], scalar1=b_sb[:, m : m + 1]
                        )
                    else:
                        nc.vector.tensor_copy(out=dst, in_=ps[:, :])
                return f

            def vtr_step(h, half=None):
                # the DVE interleave copy into v_aug layout is pushed as a
                # deferred unit so its wait on the (slow) XBAR transpose
                # never blocks the DVE FIFO
                lo = 0 if half in (None, 0) else T // 2
                hi = T if half in (None, 1) else T // 2
                klo, khi = lo // 128, hi // 128

                def f():
                    vscr = vscrp.tile(
                        [128, khi - klo, 64], BF16, name="vscr", tag=f"vs{h}"
                    )
                    nc.sync.dma_start_transpose(
                        out=vscr[:, :, :],
                        in_=vt_b[64 * h : 64 * h + 64, lo:hi],
                    )

                    def copy_unit():
                        nc.vector.tensor_copy(
                            out=v_bufs[bb % 2][:, klo:khi, 65 * h : 65 * h + 64],
                            in_=vscr[:, :, :],
                        )
                    units.append(copy_unit)
                return f

            steps = [dma_step(0), dma_step(1)]
            for rcl in range(T // RC):
                for m in range(3):
                    steps.append(mm_step(rcl, m))
                if rcl + 2 < T // RC:
                    # keep the x_t load one full row-chunk ahead of its use
                    steps.append(dma_step(rcl + 2))
                if bb == 0 and rcl in (1, 3):
                    # batch 0 runs un-interleaved: transpose each V half as
                    # soon as it exists so qc0's AV is not gated at the end
                    steps.append(vtr_step(0, rcl // 2))
                    steps.append(vtr_step(1, rcl // 2))
            if bb != 0:
                steps.append(vtr_step(0))
                steps.append(vtr_step(1))
            return {"qt": qt, "ktt": ktt}, steps

        units = deque()  # movable small work units (oproj, deferred muls)

        # prologue: batch 0 stage A runs un-interleaved
        tiles0, steps0 = make_stage_a(0)
        for st in steps0:
            st()
        while units:  # batch 0's v-interleave copies, needed by qc0's AV
            units.popleft()()
        tiles = {0: tiles0}

        for b in range(B):
            obase = b * T
            v_sb = v_bufs[b % 2]
            qt, ktt = tiles[b]["qt"], tiles[b]["ktt"]
            sa_steps = deque()
            if b + 1 < B:
                tiles[b + 1], st = make_stage_a(b + 1)
                sa_steps.extend(st)

            toggle = [0]

            def pump():
                # alternate between stage-A steps and movable units so neither
                # category bunches up (units must also drain within a few
                # slots of their push to stay ahead of tile-ring recycling)
                toggle[0] ^= 1
                if toggle[0] and sa_steps:
                    sa_steps.popleft()()
                elif units:
                    units.popleft()()
                elif sa_steps:
                    sa_steps.popleft()()

            o_sb = osbp.tile([128, T], BF16, name="o_sb", tag="osb")
            for qc in range(NQC):
                o_ps = [
                    psO.tile([65, QC], F32, name=f"o_ps{h}", tag=f"o{h}")
                    for h in range(2)
                ]
                nkb = 4 * qc + 4

                def emit_v(kb, off, n, p_t):
                    for h in range(2):
                        nc.tensor.matmul(
                            o_ps[h][:, off:QC],
                            lhsT=v_sb[:, kb, 65 * h : 65 * h + 65],
                            rhs=p_t[:, h, 0:n],
                            start=(kb == 0),
                            stop=(kb == nkb - 1),
                            skip_group_check=True,
                        )

                pending = []
                for kb in range(nkb):
                    off = max(0, (kb - 4 * qc) * 128)
                    n = QC - off
                    s_ps = psS.tile([128, 2, QC], F32, name="s_ps", tag="s")
                    for h in range(2):
                        nc.tensor.matmul(
                            s_ps[:, h, 0:n],
                            lhsT=ktt[
                                64 * h : 64 * h + 64,
                                kb * 128 : (kb + 1) * 128,
                            ],
                            rhs=qt[
                                64 * h : 64 * h + 64,
                                qc * QC + off : (qc + 1) * QC,
                            ],
                            start=True,
                            stop=True,
                        )
                    # one exp for both heads: frees both heads' score slots at
                    # the same instant so the next S pair row-packs on the PE
                    p_t = ptp.tile([128, 2, QC], BF16, name="p_t", tag="pt")
                    nc.scalar.activation(
                        out=p_t[:, :, 0:n], in_=s_ps[:, :, 0:n], func=Exp
                    )
                    if kb >= 4 * qc:
                        # diagonal block: upper-tri (q >= k) keep mask
                        nc.vector.tensor_mul(
                            out=p_t[:, :, 0:128],
                            in0=p_t[:, :, 0:128],
                            in1=tri_sb[:, :].unsqueeze(1).broadcast_to([128, 2, 128]),
                        )
                    # V-matmuls run a few steps behind so their exp is already
                    # done when they reach the head of the PE FIFO
                    pending.append((kb, off, n, p_t))
                    if len(pending) > 3:
                        emit_v(*pending.pop(0))
                    # two pumps per kb: drains stage A well before the batch
                    # boundary (pump is a no-op once everything is emitted)
                    pump()
                    pump()
                for pv in pending:
                    emit_v(*pv)

                # evacuate both accumulators immediately (frees the o_ps banks
                # before any divide-chain op can block the DVE queue)
                onum = [
                    onump.tile([65, QC], F32, name=f"onum{h}", tag=f"onum{h}")
                    for h in range(2)
                ]
                for h in range(2):
                    nc.vector.tensor_copy(out=onum[h][:, :], in_=o_ps[h][:, :])
                # reciprocal with lanes spread 1x512 -> 16x32 (recip cost
                # scales with free-size per lane); both heads' chains emitted
                # stage-by-stage so no DVE op waits behind a slow gpsimd dep
                sp16 = [
                    smallp.tile([16, 32], F32, name="sp16", tag=f"sp16{h}")
                    for h in range(2)
                ]
                srow = [
                    smallp.tile([1, QC], F32, name="srow0", tag=f"sr0{h}")
                    for h in range(2)
                ]
                bch = [
                    smallp.tile([64, QC], F32, name="bch", tag=f"bc{h}")
                    for h in range(2)
                ]
                for h in range(2):
                    nc.gpsimd.dma_start(out=sp16[h][:, :], in_=onum[h][64:65, :])
                for h in range(2):
                    nc.vector.reciprocal(out=sp16[h][:, :], in_=sp16[h][:, :])
                for h in range(2):
                    nc.gpsimd.dma_start(out=srow[h][0:1, :], in_=sp16[h][:, :])
                for h in range(2):
                    nc.gpsimd.partition_broadcast(
                        out_ap=bch[h][:, :], in_ap=srow[h][0:1, :]
                    )

                def mul_unit(qcc, on, bc, osb):
                    # divides stay on DVE (gpsimd tensor ops would force a Q7
                    # library swap against partition_broadcast, ~6us each);
                    # they are deferred units so by the time they enter the
                    # DVE FIFO the broadcast they wait on is already done
                    def f0():
                        nc.vector.tensor_mul(
                            out=osb[0:64, qcc * QC : (qcc + 1) * QC],
                            in0=on[0][0:64, :],
                            in1=bc[0][:, :],
                        )

                    def f1():
                        htmp = smallp.tile(
                            [64, QC], BF16, name="htmp", tag="htmp"
                        )
                        nc.vector.tensor_mul(
                            out=htmp[:, :], in0=on[1][0:64, :], in1=bc[1][:, :]
                        )
                        # lane shift h1 dims to partitions 64:128 via DMA
                        nc.gpsimd.dma_start(
                            out=osb[64:128, qcc * QC : (qcc + 1) * QC],
                            in_=htmp[:, :],
                        )
                    return [f0, f1]

                units.extend(mul_unit(qc, onum, bch, o_sb))

                def oproj_unit(ob, osb, rb):
                    def f():
                        # y partials ship as bf16 (the host sums in f32):
                        # halves the store bytes, and both 512-wide chunks of
                        # a row block leave in a single DMA so the y burst
                        # takes half the ring slots
                        yst = ystp.tile([128, C], BF16, name="yst", tag="yst")
                        for j in range(C // 512):
                            yps = psA.tile(
                                [128, 512], F32, name="yps", tag="qkv"
                            )
                            nc.tensor.matmul(
                                yps[:, :],
                                lhsT=osb[:, rb * 128 : (rb + 1) * 128],
                                rhs=wo_sb[:, j * 512 : (j + 1) * 512],
                                start=True,
                                stop=True,
                            )
                            nc.vector.tensor_copy(
                                out=yst[:, j * 512 : (j + 1) * 512],
                                in_=yps[:, :],
                            )
                        nc.sync.dma_start(
                            out=y[ob + rb * 128 : ob + (rb + 1) * 128, :],
                            in_=yst[:, :],
                        )
                    return f

                for rb in range(qc * 4, qc * 4 + 4):
                    units.append(oproj_unit(obase, o_sb, rb))

            # stage A of b+1 must be fully emitted before attention(b+1)
            # reads its tiles
            while sa_steps:
                sa_steps.popleft()()
        while units:
            units.popleft()()


def _build(use_bias=True):
    nc = bacc.Bacc("TRN2", target_bir_lowering=False)
    xt = nc.dram_tensor("xt", [C, R], BF16, kind="ExternalInput")
    wqkv = nc.dram_tensor("wqkv", [C, 3 * HD], BF16, kind="ExternalInput")
    bqkv = nc.dram_tensor("bqkv", [3, HD], F32, kind="ExternalInput")
    wo = nc.dram_tensor("wo", [HD, C], BF16, kind="ExternalInput")
    tri = nc.dram_tensor("tri", [128, 128], BF16, kind="ExternalInput")
    y = nc.dram_tensor("y", [R, C], BF16, kind="ExternalOutput")
    with tile.TileContext(nc) as tc:
        _emit(nc, tc, xt, wqkv, bqkv, wo, tri, y, use_bias)
    nc.finalize()
    return nc


def kernel(hidden_states, w_qkv, b_qkv, w_o, b_o):
    global LAST_RESULT, _CACHED_NC
    X = np.ascontiguousarray(np.asarray(hidden_states, dtype=np.float32)).reshape(
        R, C
    )
    w_qkv = np.asarray(w_qkv, dtype=np.float32)
    b_qkv = np.asarray(b_qkv, dtype=np.float32)
    w_o = np.asarray(w_o, dtype=np.float32)
    b_o = np.asarray(b_o, dtype=np.float32)

    bf = ml_dtypes.bfloat16
    Xt = np.ascontiguousarray(X.T).astype(bf)  # [C, R]
    scale = float(DH) ** -0.5
    tri_m = np.triu(np.ones((128, 128), dtype=np.float32)).astype(bf)

    in_maps = []
    for c in range(NCORES):
        heads = [HPC * c + i for i in range(HPC)]
        wcols, bcols = [], []
        for sec in range(3):  # q, k, v
            sc = scale if sec == 0 else 1.0
            for h in heads:
                lo = sec * C + h * DH
                wcols.append(w_qkv[:, lo : lo + DH] * sc)
                bcols.append(b_qkv[lo : lo + DH] * sc)
        wqkv_c = np.ascontiguousarray(np.concatenate(wcols, axis=1)).astype(bf)
        bqkv_c = np.ascontiguousarray(np.concatenate(bcols).reshape(3, HD))
        wo_c = np.ascontiguousarray(
            np.concatenate([w_o[h * DH : (h + 1) * DH, :] for h in heads], axis=0)
        ).astype(bf)  # [HD, C]
        in_maps.append(
            {
                "xt": Xt,
                "wqkv": wqkv_c,
                "bqkv": bqkv_c,
                "wo": wo_c,
                "tri": tri_m,
            }
        )

    if _CACHED_NC is None:
        _CACHED_NC = _build(use_bias=bool(np.any(b_qkv)))
    res = run_bass_kernel_spmd(_CACHED_NC, in_maps, core_ids=list(range(NCORES)))
    LAST_RESULT = res

    out = res.results[0]["y"].astype(np.float64)
    for c in range(1, NCORES):
        out += res.results[c]["y"]
    out += b_o
    return out.astype(np.float32).reshape(B, T, C)


# revision 29
# speedup vs baseline: 1.0259x; 1.0259x over previous
"""Causal multi-head attention block (qkv proj + attention + out proj) on 8
Trainium2 NeuronCores.

Sharding: Megatron-style tensor parallel over heads — 2 heads per core.
Each core computes its heads' Q/K/V projections (column-sharded w_qkv),
causal attention for those heads, and a row-sharded partial of the output
projection.  The host sums the 8 partial outputs and adds b_o.

Device-side layout notes:
 - The host feeds X^T [C, B*T] (bf16) so every matmul contraction dim (C,
   head dim, or key position) lands on SBUF partitions with no on-device
   transposes of activations.  Scores are computed transposed
   (S^T[k, q] = K^T.T @ Q^T per 128-wide k block), softmax-exp runs on the
   scalar engine, and the denominator is produced by an extra all-ones
   column appended to V (row 64 of the attn@V accumulator).
 - V^T -> V goes through the XBAR dma transpose path (16x128 tiles) into a
   contiguous scratch, then one DVE copy interleaves each head into the
   [V_h0 | 1 | V_h1 | 1] augmented layout.  The ones columns are memset
   once into two persistent v buffers (manual double buffer across batches).
 - The whole kernel is software-pipelined at emission level: the attention
   phase of batch b is exp(ACT)-paced, so stage A (qkv projection) of batch
   b+1, the output projection, and the softmax-divide muls are chopped into
   small units and dealt one per kb-block into the attention instruction
   stream.  This keeps the in-order PE queue fed (no engine ever waits
   long on a cross-engine dependency) and the PE clock un-throttled.
"""

import numpy as np
import ml_dtypes
from collections import deque

import concourse.bass as bass
import concourse.tile as tile
import concourse.mybir as mybir
from concourse import bacc
from concourse.bass_utils import run_bass_kernel_spmd

B, T, C, H, DH = 4, 2048, 1024, 16, 64
NCORES = 8
HPC = H // NCORES            # heads per core = 2
R = B * T                    # 8192 rows
HD = HPC * DH                # 128 local head dims
KT = C // 128                # 8 contraction tiles over C
RC = 512                     # row chunk in qkv stage
QC = 512                     # query chunk in attention
NQC = T // QC                # 4
NKB = T // 128               # 16 key blocks

F32 = mybir.dt.float32
BF16 = mybir.dt.bfloat16

LAST_RESULT = None           # BassKernelResults of the most recent run
_CACHED_NC = None


def _emit(nc, tc, xt, wqkv, bqkv, wo, tri, y, use_bias=True):
    from contextlib import ExitStack

    Exp = mybir.ActivationFunctionType.Exp
    with ExitStack() as ctx:
        const = ctx.enter_context(tc.tile_pool(name="const", bufs=1))
        bigp = ctx.enter_context(tc.tile_pool(name="bigp", bufs=2))
        xtp = ctx.enter_context(tc.tile_pool(name="xtp", bufs=4))
        vtp = ctx.enter_context(tc.tile_pool(name="vtp", bufs=2))
        vscrp = ctx.enter_context(tc.tile_pool(name="vscrp", bufs=2))
        ptp = ctx.enter_context(tc.tile_pool(name="ptp", bufs=7))
        osbp = ctx.enter_context(tc.tile_pool(name="osbp", bufs=2))
        ystp = ctx.enter_context(tc.tile_pool(name="ystp", bufs=4))
        smallp = ctx.enter_context(tc.tile_pool(name="smallp", bufs=3))
        onump = ctx.enter_context(tc.tile_pool(name="onump", bufs=3))
        psA = ctx.enter_context(tc.tile_pool(name="psA", bufs=2, space="PSUM"))
        psS = ctx.enter_context(tc.tile_pool(name="psS", bufs=2, space="PSUM"))
        psO = ctx.enter_context(tc.tile_pool(name="psO", bufs=1, space="PSUM"))

        # ---- constants ----
        w_sb = const.tile([128, KT, 3 * HD], BF16, name="w_sb")
        wqkv_r = wqkv[:, :].rearrange("(ko ki) m -> ki ko m", ki=128)
        for m in range(3):
            # split per q/k/v section so the first qkv matmul group only
            # waits on a third of the weight bytes
            nc.sync.dma_start(
                out=w_sb[:, :, m * HD : (m + 1) * HD],
                in_=wqkv_r[:, :, m * HD : (m + 1) * HD],
            )
        wo_sb = const.tile([128, C], BF16, name="wo_sb")
        nc.sync.dma_start(out=wo_sb[:, :], in_=wo[:, :])
        b_sb = const.tile([128, 3], F32, name="b_sb")
        for m in range(3):
            nc.sync.dma_start(
                out=b_sb[:, m : m + 1],
                in_=bqkv[m : m + 1, :].rearrange("a n -> n a"),
            )
        tri_sb = const.tile([128, 128], BF16, name="tri_sb")
        nc.sync.dma_start(out=tri_sb[:, :], in_=tri[:, :])

        # persistent double-buffered V_aug tiles; ones columns written once
        v_bufs = [
            const.tile([128, NKB, 130], BF16, name=f"v_sb{i}") for i in range(2)
        ]
        for vb in v_bufs:
            nc.vector.memset(vb[:, :, 64:65], 1.0)
            nc.vector.memset(vb[:, :, 129:130], 1.0)

        # warm up the ACT exp table while stage A of batch 0 runs
        warm = const.tile([1, 1], F32, name="warm")
        nc.vector.memset(warm[:, :], 0.0)
        nc.scalar.activation(out=warm[:, :], in_=warm[:, :], func=Exp)

        xt_r = xt[:, :].rearrange("(ko ki) n -> ki ko n", ki=128)

        # ---- stage A of one batch, chopped into pump-able steps ----
        def make_stage_a(bb):
            qt = bigp.tile([128, T], BF16, name="qt", tag="qt")
            ktt = bigp.tile([128, T], BF16, name="ktt", tag="ktt")
            vt_b = vtp.tile([128, T], BF16, name="vt_b", tag="vt")
            xts = {}

            def dma_step(rcl):
                def f():
                    x_t = xtp.tile([128, KT, RC], BF16, name="x_t", tag="xt")
                    rc = bb * (T // RC) + rcl
                    nc.sync.dma_start(
                        out=x_t[:, :, :],
                        in_=xt_r[:, :, rc * RC : (rc + 1) * RC],
                    )
                    xts[rcl] = x_t
                return f

            def mm_step(rcl, m):
                def f():
                    ps = psA.tile([128, RC], F32, name="ps_qkv", tag="qkv")
                    for k in range(KT):
                        nc.tensor.matmul(
                            ps[:, :],
                            lhsT=w_sb[:, k, m * HD : (m + 1) * HD],
                            rhs=xts[rcl][:, k, :],
                            start=(k == 0),
                            stop=(k == KT - 1),
                        )
                    if m == 0:
                        dst = qt[:, rcl * RC : (rcl + 1) * RC]
                    elif m == 1:
                        dst = ktt[:, rcl * RC : (rcl + 1) * RC]
                    else:
                        dst = vt_b[:, rcl * RC : (rcl + 1) * RC]
                    if use_bias:
                        nc.vector.tensor_scalar_add(
                            out=dst, in0=ps[:, :], scalar1=b_sb[:, m : m + 1]
                        )
                    else:
                        nc.vector.tensor_copy(out=dst, in_=ps[:, :])
                return f

            def vtr_step(h, half=None):
                # the DVE interleave copy into v_aug layout is pushed as a
                # deferred unit so its wait on the (slow) XBAR transpose
                # never blocks the DVE FIFO
                lo = 0 if half in (None, 0) else T // 2
                hi = T if half in (None, 1) else T // 2
                klo, khi = lo // 128, hi // 128

                def f():
                    vscr = vscrp.tile(
                        [128, khi - klo, 64], BF16, name="vscr", tag=f"vs{h}"
                    )
                    nc.sync.dma_start_transpose(
                        out=vscr[:, :, :],
                        in_=vt_b[64 * h : 64 * h + 64, lo:hi],
                    )

                    def copy_unit():
                        nc.vector.tensor_copy(
                            out=v_bufs[bb % 2][:, klo:khi, 65 * h : 65 * h + 64],
                            in_=vscr[:, :, :],
                        )
                    units.append(copy_unit)
                return f

            steps = [dma_step(0), dma_step(1)]
            for rcl in range(T // RC):
                for m in range(3):
                    steps.append(mm_step(rcl, m))
                if rcl + 2 < T // RC:
                    # keep the x_t load one full row-chunk ahead of its use
                    steps.append(dma_step(rcl + 2))
                if bb == 0 and rcl in (1, 3):
                    # batch 0 runs un-interleaved: transpose each V half as
                    # soon as it exists so qc0's AV is not gated at the end
                    steps.append(vtr_step(0, rcl // 2))
                    steps.append(vtr_step(1, rcl // 2))
            if bb != 0:
                steps.append(vtr_step(0))
                steps.append(vtr_step(1))
            return {"qt": qt, "ktt": ktt}, steps

        units = deque()  # movable small work units (oproj, deferred muls)

        # prologue: batch 0 stage A runs un-interleaved
        tiles0, steps0 = make_stage_a(0)
        for st in steps0:
            st()
        while units:  # batch 0's v-interleave copies, needed by qc0's AV
            units.popleft()()
        tiles = {0: tiles0}

        for b in range(B):
            obase = b * T
            v_sb = v_bufs[b % 2]
            qt, ktt = tiles[b]["qt"], tiles[b]["ktt"]
            sa_steps = deque()
            if b + 1 < B:
                tiles[b + 1], st = make_stage_a(b + 1)
                sa_steps.extend(st)

            def pump():
                # stage A first: it (and the V transposes at its end) then
                # finishes mid-batch, far from the batch-boundary DMA burst;
                # oproj/divide units fill the remaining kb slots, which also
                # keeps them well within the tile-ring recycling bounds
                if sa_steps:
                    sa_steps.popleft()()
                elif units:
                    units.popleft()()

            o_sb = osbp.tile([128, T], BF16, name="o_sb", tag="osb")
            for qc in range(NQC):
                o_ps = [
                    psO.tile([65, QC], F32, name=f"o_ps{h}", tag=f"o{h}")
                    for h in range(2)
                ]
                nkb = 4 * qc + 4

                def emit_v(kb, off, n, p_t):
                    for h in range(2):
                        nc.tensor.matmul(
                            o_ps[h][:, off:QC],
                            lhsT=v_sb[:, kb, 65 * h : 65 * h + 65],
                            rhs=p_t[:, h, 0:n],
                            start=(kb == 0),
                            stop=(kb == nkb - 1),
                            skip_group_check=True,
                        )

                pending = []
                for kb in range(nkb):
                    off = max(0, (kb - 4 * qc) * 128)
                    n = QC - off
                    s_ps = psS.tile([128, 2, QC], F32, name="s_ps", tag="s")
                    for h in range(2):
                        nc.tensor.matmul(
                            s_ps[:, h, 0:n],
                            lhsT=ktt[
                                64 * h : 64 * h + 64,
                                kb * 128 : (kb + 1) * 128,
                            ],
                            rhs=qt[
                                64 * h : 64 * h + 64,
                                qc * QC + off : (qc + 1) * QC,
                            ],
                            start=True,
                            stop=True,
                        )
                    # one exp for both heads: frees both heads' score slots at
                    # the same instant so the next S pair row-packs on the PE
                    p_t = ptp.tile([128, 2, QC], BF16, name="p_t", tag="pt")
                    nc.scalar.activation(
                        out=p_t[:, :, 0:n], in_=s_ps[:, :, 0:n], func=Exp
                    )
                    if kb >= 4 * qc:
                        # diagonal block: upper-tri (q >= k) keep mask
                        nc.vector.tensor_mul(
                            out=p_t[:, :, 0:128],
                            in0=p_t[:, :, 0:128],
                            in1=tri_sb[:, :].unsqueeze(1).broadcast_to([128, 2, 128]),
                        )
                    # V-matmuls run a few steps behind so their exp is already
                    # done when they reach the head of the PE FIFO
                    pending.append((kb, off, n, p_t))
                    if len(pending) > 3:
                        emit_v(*pending.pop(0))
                    # two pumps per kb: drains stage A well before the batch
                    # boundary (pump is a no-op once everything is emitted)
                    pump()
                    pump()
                for pv in pending:
                    emit_v(*pv)

                # evacuate both accumulators immediately (frees the o_ps banks
                # before any divide-chain op can block the DVE queue)
                onum = [
                    onump.tile([65, QC], F32, name=f"onum{h}", tag=f"onum{h}")
                    for h in range(2)
                ]
                for h in range(2):
                    nc.vector.tensor_copy(out=onum[h][:, :], in_=o_ps[h][:, :])
                # reciprocal with lanes spread 1x512 -> 16x32 (recip cost
                # scales with free-size per lane); both heads' chains emitted
                # stage-by-stage so no DVE op waits behind a slow gpsimd dep
                sp16 = [
                    smallp.tile([16, 32], F32, name="sp16", tag=f"sp16{h}")
                    for h in range(2)
                ]
                srow = [
                    smallp.tile([1, QC], F32, name="srow0", tag=f"sr0{h}")
                    for h in range(2)
                ]
                bch = [
                    smallp.tile([64, QC], F32, name="bch", tag=f"bc{h}")
                    for h in range(2)
                ]
                for h in range(2):
                    nc.gpsimd.dma_start(out=sp16[h][:, :], in_=onum[h][64:65, :])
                for h in range(2):
                    nc.vector.reciprocal(out=sp16[h][:, :], in_=sp16[h][:, :])
                for h in range(2):
                    nc.gpsimd.dma_start(out=srow[h][0:1, :], in_=sp16[h][:, :])
                for h in range(2):
                    nc.gpsimd.partition_broadcast(
                        out_ap=bch[h][:, :], in_ap=srow[h][0:1, :]
                    )

                def mul_unit(qcc, on, bc, osb):
                    # divides stay on DVE (gpsimd tensor ops would force a Q7
                    # library swap against partition_broadcast, ~6us each);
                    # they are deferred units so by the time they enter the
                    # DVE FIFO the broadcast they wait on is already done
                    def f0():
                        nc.vector.tensor_mul(
                            out=osb[0:64, qcc * QC : (qcc + 1) * QC],
                            in0=on[0][0:64, :],
                            in1=bc[0][:, :],
                        )

                    def f1():
                        htmp = smallp.tile(
                            [64, QC], BF16, name="htmp", tag="htmp"
                        )
                        nc.vector.tensor_mul(
                            out=htmp[:, :], in0=on[1][0:64, :], in1=bc[1][:, :]
                        )
                        # lane shift h1 dims to partitions 64:128 via DMA
                        nc.gpsimd.dma_start(
                            out=osb[64:128, qcc * QC : (qcc + 1) * QC],
                            in_=htmp[:, :],
                        )
                    return [f0, f1]

                units.extend(mul_unit(qc, onum, bch, o_sb))

                def oproj_unit(ob, osb, rb):
                    def f():
                        # y partials ship as bf16 (the host sums in f32):
                        # halves the store bytes, and both 512-wide chunks of
                        # a row block leave in a single DMA so the y burst
                        # takes half the ring slots
                        yst = ystp.tile([128, C], BF16, name="yst", tag="yst")
                        for j in range(C // 512):
                            yps = psA.tile(
                                [128, 512], F32, name="yps", tag="qkv"
                            )
                            nc.tensor.matmul(
                                yps[:, :],
                                lhsT=osb[:, rb * 128 : (rb + 1) * 128],
                                rhs=wo_sb[:, j * 512 : (j + 1) * 512],
                                start=True,
                                stop=True,
                            )
                            nc.vector.tensor_copy(
                                out=yst[:, j * 512 : (j + 1) * 512],
                                in_=yps[:, :],
                            )
                        nc.sync.dma_start(
                            out=y[ob + rb * 128 : ob + (rb + 1) * 128, :],
                            in_=yst[:, :],
                        )
                    return f

                for rb in range(qc * 4, qc * 4 + 4):
                    units.append(oproj_unit(obase, o_sb, rb))

            # stage A of b+1 must be fully emitted before attention(b+1)
            # reads its tiles
            while sa_steps:
                sa_steps.popleft()()
        while units:
            units.popleft()()


def _build(use_bias=True):
    nc = bacc.Bacc("TRN2", target_bir_lowering=False)
    xt = nc.dram_tensor("xt", [C, R], BF16, kind="ExternalInput")
    wqkv = nc.dram_tensor("wqkv", [C, 3 * HD], BF16, kind="ExternalInput")
    bqkv = nc.dram_tensor("bqkv", [3, HD], F32, kind="ExternalInput")
    wo = nc.dram_tensor("wo", [HD, C], BF16, kind="ExternalInput")
    tri = nc.dram_tensor("tri", [128, 128], BF16, kind="ExternalInput")
    y = nc.dram_tensor("y", [R, C], BF16, kind="ExternalOutput")
    with tile.TileContext(nc) as tc:
        _emit(nc, tc, xt, wqkv, bqkv, wo, tri, y, use_bias)
    nc.finalize()
    return nc


def kernel(hidden_states, w_qkv, b_qkv, w_o, b_o):
    global LAST_RESULT, _CACHED_NC
    X = np.ascontiguousarray(np.asarray(hidden_states, dtype=np.float32)).reshape(
        R, C
    )
    w_qkv = np.asarray(w_qkv, dtype=np.float32)
    b_qkv = np.asarray(b_qkv, dtype=np.float32)
    w_o = np.asarray(w_o, dtype=np.float32)
    b_o = np.asarray(b_o, dtype=np.float32)

    bf = ml_dtypes.bfloat16
    Xt = np.ascontiguousarray(X.T).astype(bf)  # [C, R]
    scale = float(DH) ** -0.5
    tri_m = np.triu(np.ones((128, 128), dtype=np.float32)).astype(bf)

    in_maps = []
    for c in range(NCORES):
        heads = [HPC * c + i for i in range(HPC)]
        wcols, bcols = [], []
        for sec in range(3):  # q, k, v
            sc = scale if sec == 0 else 1.0
            for h in heads:
                lo = sec * C + h * DH
                wcols.append(w_qkv[:, lo : lo + DH] * sc)
                bcols.append(b_qkv[lo : lo + DH] * sc)
        wqkv_c = np.ascontiguousarray(np.concatenate(wcols, axis=1)).astype(bf)
        bqkv_c = np.ascontiguousarray(np.concatenate(bcols).reshape(3, HD))
        wo_c = np.ascontiguousarray(
            np.concatenate([w_o[h * DH : (h + 1) * DH, :] for h in heads], axis=0)
        ).astype(bf)  # [HD, C]
        in_maps.append(
            {
                "xt": Xt,
                "wqkv": wqkv_c,
                "bqkv": bqkv_c,
                "wo": wo_c,
                "tri": tri_m,
            }
        )

    if _CACHED_NC is None:
        _CACHED_NC = _build(use_bias=bool(np.any(b_qkv)))
    res = run_bass_kernel_spmd(_CACHED_NC, in_maps, core_ids=list(range(NCORES)))
    LAST_RESULT = res

    out = res.results[0]["y"].astype(np.float64)
    for c in range(1, NCORES):
        out += res.results[c]["y"]
    out += b_o
    return out.astype(np.float32).reshape(B, T, C)
